# revision 4
# baseline (speedup 1.0000x reference)
"""F1-score (histogram_binning) Trainium2 Bass kernel — mask formulation.

The reference F1 epilogue only consumes diag(cm), cm[:,0], cm[:,1],
cm[0,:], cm[1,:] — not the full confusion matrix. Those reduce to three
per-sample boolean masks plus tiny label bincounts:

  match[s] = (x[s, y_true[s]] >= rowmax[s])   <=>  pred == true
  p0[s]    = (x[s, 0]        >= rowmax[s])    <=>  pred == 0   (exact:
             argmax is first-max, so x[s,0]==max always means pred 0)
  p1[s]    = (x[s, 1] >= rowmax[s]) & ~p0[s]  <=>  pred == 1

Device work per core (memory-bound, ~64 MiB y_pred stream):
  - 64 blocks of [128 part x 16 samp x 128 cls] fp32 via one HWDGE queue
  - VectorE: rowmax tensor_reduce per block + two tiny strided is_ge TTs
    (columns 0/1 of each sample row) per block; per 16-block chunk one
    is_ge of host-gathered x_true vs rowmax
  - masks accumulate in SBUF, one 768 KB bf16 store at the end
No one-hots, no matmuls, no ScalarE work: DMA is the only near-saturated
engine. Host: bincounts of y_true over the masks, argmax of the ~16k rows
with true<=1 (rows 0/1 of cm), then the exact fp32 F1 epilogue.
"""

import sys

import numpy as np

sys.path.insert(0, "/opt/trn_rl_repo")

import concourse.bacc as bacc  # noqa: E402
import concourse.tile as tile  # noqa: E402
from concourse import mybir  # noqa: E402
from concourse.bass_utils import run_bass_kernel_spmd  # noqa: E402

N_CORES = 8
N_SAMPLES = 1048576
C = 128
EPS = 1e-07
N_PER_CORE = N_SAMPLES // N_CORES  # 131072
P = 128  # partitions
F_PER_PART = N_PER_CORE // P  # 1024 samples per partition
G = 16  # samples per partition per block
N_BLOCKS = F_PER_PART // G  # 64 blocks of 1 MiB
CHUNK = 16  # blocks per match-TT / 256 samples per partition
N_CHUNKS = N_BLOCKS // CHUNK


def build_program():
    nc = bacc.Bacc("TRN2")

    y_pred = nc.dram_tensor(
        "y_pred", [N_PER_CORE, C], mybir.dt.float32, kind="ExternalInput"
    )
    # x_true[p, t] = y_pred_local[p*1024 + t, y_true[p*1024 + t]] (host gather)
    x_true = nc.dram_tensor(
        "x_true", [P, F_PER_PART], mybir.dt.float32, kind="ExternalInput"
    )
    # masks[p, 0, :]=match, [p, 1, :]=pred0, [p, 2, :]=pred1-ish (x1>=max)
    masks_t = nc.dram_tensor(
        "masks", [P, 3, F_PER_PART], mybir.dt.bfloat16, kind="ExternalOutput"
    )

    # sample s_local = p * F_PER_PART + b*G + g -> contiguous per-partition DMA
    xs = y_pred[:].rearrange("(p b g) c -> p b g c", p=P, b=N_BLOCKS, g=G)

    with tile.TileContext(nc) as tc:
        with (
            tc.tile_pool(name="consts", bufs=1) as consts,
            tc.tile_pool(name="xp", bufs=10) as xp,
        ):
            xt_sb = consts.tile([P, F_PER_PART], mybir.dt.float32, tag="xt")
            nc.gpsimd.dma_start(out=xt_sb, in_=x_true[:])

            rm_all = consts.tile([P, F_PER_PART], mybir.dt.float32, tag="rm")
            # columns 0/1 of each sample row, extracted by ScalarE
            x01_all = consts.tile([P, 2, F_PER_PART], mybir.dt.float32, tag="x01")
            mk_all = consts.tile([P, 3, F_PER_PART], mybir.dt.bfloat16, tag="mk")

            for b in range(N_BLOCKS):
                x_t = xp.tile([P, G, C], mybir.dt.float32)
                # alternate the two HWDGE rings for descriptor supply
                dma_eng = nc.sync if b % 2 == 0 else nc.scalar
                dma_eng.dma_start(out=x_t, in_=xs[:, b])

                sl = slice(b * G, (b + 1) * G)
                nc.vector.tensor_reduce(
                    out=rm_all[:, sl],
                    in_=x_t,
                    axis=mybir.AxisListType.X,
                    op=mybir.AluOpType.max,
                )
                # strided column extraction on the (otherwise idle) ScalarE
                nc.scalar.copy(out=x01_all[:, 0, sl], in_=x_t[:, :, 0])
                nc.scalar.copy(out=x01_all[:, 1, sl], in_=x_t[:, :, 1])

                if b % CHUNK == CHUNK - 1:
                    k = b // CHUNK
                    ck = slice(k * CHUNK * G, (k + 1) * CHUNK * G)
                    nc.vector.tensor_tensor(
                        out=mk_all[:, 0, ck],
                        in0=xt_sb[:, ck],
                        in1=rm_all[:, ck],
                        op=mybir.AluOpType.is_ge,
                    )
                    nc.vector.tensor_tensor(
                        out=mk_all[:, 1, ck],
                        in0=x01_all[:, 0, ck],
                        in1=rm_all[:, ck],
                        op=mybir.AluOpType.is_ge,
                    )
                    nc.vector.tensor_tensor(
                        out=mk_all[:, 2, ck],
                        in0=x01_all[:, 1, ck],
                        in1=rm_all[:, ck],
                        op=mybir.AluOpType.is_ge,
                    )
                    # stream this chunk's masks out on the SWDGE queue so the
                    # kernel tail is only the last chunk's small store
                    nc.gpsimd.dma_start(
                        out=masks_t[:, :, ck], in_=mk_all[:, :, ck]
                    )

    nc.finalize()
    return nc


_PROGRAM = None


def _get_program():
    global _PROGRAM
    if _PROGRAM is None:
        _PROGRAM = build_program()
    return _PROGRAM


def _shard_inputs(y_pred, y_true):
    y_pred = np.ascontiguousarray(np.asarray(y_pred), dtype=np.float32)
    y_true = np.asarray(y_true).astype(np.int64)
    x_true_full = np.take_along_axis(y_pred, y_true[:, None], axis=1)[:, 0]
    in_maps = []
    for c in range(N_CORES):
        sl = slice(c * N_PER_CORE, (c + 1) * N_PER_CORE)
        in_maps.append(
            {
                "y_pred": y_pred[sl],
                "x_true": np.ascontiguousarray(
                    x_true_full[sl].reshape(P, F_PER_PART)
                ),
            }
        )
    return in_maps


def _assemble(y_pred, y_true, match, p0, p1):
    """Exact F1 from masks + tiny host bincounts (validated vs reference)."""
    y_true = np.asarray(y_true).astype(np.int64)
    pred1 = p1 & ~p0  # exact pred==1 even under 0-1 ties
    TP = np.bincount(y_true[match], minlength=C).astype(np.float32)
    col0 = np.bincount(y_true[p0], minlength=C).astype(np.float32)
    col1 = np.bincount(y_true[pred1], minlength=C).astype(np.float32)
    sel = y_true <= 1
    pred_sel = np.argmax(y_pred[sel], axis=1)
    t_sel = y_true[sel]
    row0 = np.bincount(pred_sel[t_sel == 0], minlength=C).astype(np.float32)
    row1 = np.bincount(pred_sel[t_sel == 1], minlength=C).astype(np.float32)

    FP = np.float32(C - 1) * col1 + col0
    FN = np.float32(C - 1) * row1 + row0
    eps = np.float32(EPS)
    sensitivity = np.mean(TP / (TP + FN + eps), dtype=np.float32)
    precision = np.mean(TP / (TP + FP + eps), dtype=np.float32)
    f1 = np.float32(2.0) * (precision * sensitivity / (precision + sensitivity + eps))
    return np.asarray(f1, dtype=np.float32)


def run_on_device(y_pred, y_true, **kwargs):
    """Run the bass kernel on 8 cores; returns (masks_tuple, results_obj)."""
    nc = _get_program()
    y_pred = np.ascontiguousarray(np.asarray(y_pred), dtype=np.float32)
    y_true = np.asarray(y_true)
    in_maps = _shard_inputs(y_pred, y_true)
    res = run_bass_kernel_spmd(nc, in_maps, core_ids=list(range(N_CORES)), **kwargs)
    parts = {0: [], 1: [], 2: []}
    for r in res.results:
        m = np.asarray(r["masks"]).astype(np.float32)  # [P, 3, F_PER_PART]
        for j in range(3):
            parts[j].append(m[:, j, :].reshape(-1))  # s_local = p*1024 + t
    match = np.concatenate(parts[0]) > 0.5
    p0 = np.concatenate(parts[1]) > 0.5
    p1 = np.concatenate(parts[2]) > 0.5
    return (match, p0, p1), res


def kernel(y_pred, y_true):
    y_pred = np.ascontiguousarray(np.asarray(y_pred), dtype=np.float32)
    (match, p0, p1), _ = run_on_device(y_pred, y_true)
    return _assemble(y_pred, y_true, match, p0, p1)
